# revision 35
# baseline (speedup 1.0000x reference)
"""Trainium2 Bass kernel for a GNN message-passing layer.

Reference computation (per batch b):
    m   = relu(h @ W1.T + b1)
    m   = relu(m @ W2.T + b2)
    msg = relu(A @ m)
    gx  = msg @ W_ih.T + b_ih ; gh = h @ W_hh.T + b_hh   (gates r,z,n)
    r = sig(gxr+ghr); z = sig(gxz+ghz); n = tanh(gxn + r*ghn)
    out = (1-z)*n + z*h
Sharding: pure data-parallel over B (B == n_cores == 8).

Numerics:
  * A streamed fp16; msg decomposed msg = u (x) s + A @ (m2 - u) with
    u ~= column means of m2 (fp16-exact), s = rowsums of fp16 A; the
    streamed residual is ~40x smaller than msg so gate matmuls run in
    fast f32r. v (x) s (v = W_ih @ u) restored via an exact hi/lo f32r
    matmul (128-padded stationary; K=4 matmuls measured 2x slower).
  * Weight rounding is a per-column systematic error that the ~1024x
    adjacency sum amplifies ~1% - so W1 and W2 are hi+lo f32r splits
    (exact) and only h/m1 DATA get f32r-rounded (dithered, safe).
Performance notes (all measured on traces):
  * One ring with the deep descriptor queue gets ~330-420GB/s; any
    competing ring starves.  sync ring = [w1, hT, all A] in need order;
    scalar ring = small blobs early + out stores late; big descriptors
    only (tiny ones melt aggregate bandwidth).
  * PE needs ~9.4us of busy time before f32r/f16 matmuls hit full rate
    (216ns per 512-col): throwaway f32 matmuls from ~8us warm it up.
  * m2 computed FEATURE-major (one stationary pair for all chunks, no
    16x f32 LDWEIGHTS) then transposed through the PE into node-major
    fp16 blocks for the msg stationary.
  * Last quarter's A is packed column-split so each 256-wide half
    finishes with its own slab: the serial sig/tanh/combine tail runs
    on half tiles and overlaps the other half's matmuls.
  * ACT sem wake-ups can lag ~2-3us behind a finishing PE; the q3
    half-split keeps the ACT queue busy so wakes stay instant.
"""

import numpy as np

B, N, H = 8, 2048, 128
NCHUNK = 512
NCH = N // NCHUNK  # 4
KBLK = N // 128    # 16

# blob G (f32r): gate weights + v-factors + W2 hi/lo split
G_WIH = 0          # [0:384)     W_ih.T
G_WHH = 384        # [384:768)   W_hh.T
G_VQ = 768         # [768:1152)  rows 0:4 = [vhi;vhi;vlo;vlo], rest 0
C_G = 1152
# blob F (f32): gate biases + u (per-partition)
F_BRZ = 0          # [0:2)
F_BIHN = 2
F_BHHN = 3
F_UP = 4
C_F = 5

_CACHE = {}


def _build_program():
    import concourse.bacc as bacc
    import concourse.tile as tile
    import concourse.mybir as mybir
    from concourse.alu_op_type import AluOpType

    f32 = mybir.dt.float32
    f32r = mybir.dt.float32r
    f16 = mybir.dt.float16
    ACT = mybir.ActivationFunctionType

    nc = bacc.Bacc("TRN2", target_bir_lowering=False, debug=False, num_devices=B)

    hT_d = nc.dram_tensor("hT", [H, N], f32r, kind="ExternalInput").ap()
    A2_d = nc.dram_tensor("A2", [NCH, KBLK // 8, H, 8 * NCHUNK], f16, kind="ExternalInput").ap()
    w1_d = nc.dram_tensor("w12hl", [H, 4 * H], f32r, kind="ExternalInput").ap()
    blg_d = nc.dram_tensor("blg", [H, C_G], f32r, kind="ExternalInput").ap()
    blf_d = nc.dram_tensor("blf", [H, C_F], f32, kind="ExternalInput").ap()
    id_d = nc.dram_tensor("ident", [H, H], f16, kind="ExternalInput").ap()
    s4_d = nc.dram_tensor("s4", [4, N], f32r, kind="ExternalInput").ap()
    out_d = nc.dram_tensor("outT", [H, N], f32, kind="ExternalOutput").ap()

    with tile.TileContext(nc) as tc:
        with (
            tc.tile_pool(name="consts", bufs=1) as cp,
            tc.tile_pool(name="big", bufs=1) as bp,
            tc.tile_pool(name="at", bufs=8) as ap_,
            tc.tile_pool(name="msgp", bufs=2) as mp,
            tc.tile_pool(name="tmp", bufs=2) as tp,
            tc.tile_pool(name="outp", bufs=2) as op_,
            tc.tile_pool(name="psum", bufs=1, space="PSUM") as pp,
        ):
            w12 = cp.tile([H, 4 * H], f32r, tag="w12")
            blg = cp.tile([H, C_G], f32r, tag="blg")
            blf = cp.tile([H, C_F], f32, tag="blf")
            ident = cp.tile([H, H], f16, tag="ident")
            dummy = cp.tile([H, 1], f32, tag="dummy")
            warm = cp.tile([H, 5 * H], f32, tag="warm")
            s4p = cp.tile([H, N], f32r, tag="s4p")
            hTr = bp.tile([H, N], f32r, tag="hTr")
            m1T = bp.tile([H, N], f32r, tag="m1T")
            m2cT = bp.tile([H, N], f16, tag="m2cT")  # feature-major (m2 - u)
            m2c = bp.tile([H, N], f16, tag="m2c")    # node-major, block k at 128k..

            wih = blg[:, G_WIH:G_WIH + 3 * H]
            whh = blg[:, G_WHH:G_WHH + 3 * H]
            vqp = blg[:, G_VQ:G_VQ + 3 * H]
            brz = blf[:, F_BRZ:F_BRZ + 2]
            bihn = blf[:, F_BIHN:F_BIHN + 1]
            bhhn = blf[:, F_BHHN:F_BHHN + 1]
            uP = blf[:, F_UP:F_UP + 1]

            # ---- PE warm-up from ~7.5us (gpsimd memset is the earliest
            # writer); ~9.4us of busy time until full matmul rate ----
            nc.gpsimd.memset(warm[:], 0.0)
            ps_w = pp.tile([H, NCHUNK], f32, tag="msg", bufs=2, name="pswarm")
            for _ in range(4):
                nc.tensor.matmul(ps_w[:], warm[:, 0:H], warm[:, H:5 * H],
                                 start=True, stop=True)

            # ---- DMA issues ----
            nc.sync.dma_start(w12[:], w1_d[:])
            nc.sync.dma_start(hTr[:], hT_d[:])
            ats = {}
            for q in range(NCH):
                for g_ in range(KBLK // 8):
                    at = ap_.tile([H, 8 * NCHUNK], f16, tag="at")
                    nc.sync.dma_start(at[:], A2_d[q, g_])
                    ats[(q, g_)] = at
            nc.scalar.dma_start(blf[:], blf_d[:])
            nc.scalar.dma_start(blg[:], blg_d[:])
            nc.scalar.dma_start(ident[:], id_d[:])
            nc.vector.memset(s4p[:].bitcast(f32), 0.0)
            nc.scalar.dma_start(s4p[0:4, :], s4_d[:])

            # ---- ACT table preload (dummy sigmoid; that table also holds
            # relu/tanh/copy) ----
            nc.vector.memset(dummy[:], 0.0)
            nc.scalar.activation(dummy[:], dummy[:], ACT.Sigmoid)

            # ---- m1T = relu(W1 @ hT), W1 split hi/lo f32r; relus
            # alternate DVE/ACT (b1 == 0 per spec) ----
            for c in range(NCH):
                sl = slice(c * NCHUNK, (c + 1) * NCHUNK)
                ps_m1 = pp.tile([H, NCHUNK], f32, tag="acc", bufs=4)
                nc.tensor.matmul(ps_m1[:], w12[:, 0:H], hTr[:, sl], start=True, stop=False)
                nc.tensor.matmul(ps_m1[:], w12[:, H:2 * H], hTr[:, sl], start=False, stop=True)
                if c % 2 == 0:
                    nc.vector.tensor_scalar_max(m1T[:, sl], ps_m1[:], 0.0)
                else:
                    nc.scalar.activation(m1T[:, sl], ps_m1[:], ACT.Relu)

            # ---- m2cT = relu(W2 @ m1) - u, FEATURE-major: W2 split hi/lo
            # f32r (exact), m1 f32r-rounded (dithered).  One fused DVE op
            # per chunk: (ps max 0) - u[p]  (b2 == 0 per spec) ----
            for c in range(NCH):
                sl = slice(c * NCHUNK, (c + 1) * NCHUNK)
                ps_m2 = pp.tile([H, NCHUNK], f32, tag="acc", bufs=4)
                nc.tensor.matmul(ps_m2[:], w12[:, 2 * H:3 * H], m1T[:, sl], start=True, stop=False)
                nc.tensor.matmul(ps_m2[:], w12[:, 3 * H:4 * H], m1T[:, sl], start=False, stop=True)
                nc.vector.tensor_scalar(m2cT[:, sl], ps_m2[:], 0.0, uP,
                                        op0=AluOpType.max, op1=AluOpType.subtract)

            # ---- transpose m2cT into node-major 128-blocks through the PE;
            # psum->SBUF copies alternate ACT/DVE ----
            for k in range(KBLK):
                kb = slice(k * H, (k + 1) * H)
                ps_t = pp.tile([H, H], f16, tag="tps", bufs=2)
                nc.tensor.transpose(ps_t[:], m2cT[:, kb], ident[:])
                if k % 2 == 0:
                    nc.vector.tensor_scalar_add(m2c[:, kb], ps_t[:], 0.0)
                else:
                    nc.scalar.copy(m2c[:, kb], ps_t[:])

            # ---- pipelined quarters ----
            def emit_msg(cols, src, name):
                # one psum accumulation over all 16 k-blocks, cols wide;
                # src(k) -> (at tile, column offset)
                ps_msg = pp.tile([H, cols], f32, tag="msg", bufs=2, name=name)
                for k in range(KBLK):
                    at, off = src(k)
                    nc.tensor.matmul(
                        ps_msg[:],
                        m2c[:, k * H:(k + 1) * H],
                        at[:, off:off + cols],
                        start=(k == 0), stop=(k == KBLK - 1),
                    )
                return ps_msg

            def emit_gates(sl, w, ps_msg, resid_dve, comb_eng, name):
                # sl: output column slice (width w)
                residT = mp.tile([H, w], f32r, tag=f"residT{w}", name=name)
                if resid_dve:
                    nc.vector.tensor_scalar_add(residT[:], ps_msg[:], 0.0)
                else:
                    nc.scalar.copy(residT[:], ps_msg[:])

                ps_ghn = pp.tile([H, w], f32, tag="acc", bufs=4)
                nc.tensor.matmul(ps_ghn[:], whh[:, 2 * H:3 * H], hTr[:, sl], start=True, stop=True)

                ps_r = pp.tile([H, w], f32, tag="acc", bufs=4)
                nc.tensor.matmul(ps_r[:], whh[:, 0:H], hTr[:, sl], start=True, stop=False)
                nc.tensor.matmul(ps_r[:], vqp[:, 0:H], s4p[:, sl], start=False, stop=False)
                nc.tensor.matmul(ps_r[:], wih[:, 0:H], residT[:], start=False, stop=True)
                r = tp.tile([H, w], f32, tag=f"r{w}")
                nc.scalar.activation(r[:], ps_r[:], ACT.Sigmoid, bias=brz[:, 0:1])

                ps_z = pp.tile([H, w], f32, tag="acc", bufs=4)
                nc.tensor.matmul(ps_z[:], whh[:, H:2 * H], hTr[:, sl], start=True, stop=False)
                nc.tensor.matmul(ps_z[:], vqp[:, H:2 * H], s4p[:, sl], start=False, stop=False)
                nc.tensor.matmul(ps_z[:], wih[:, H:2 * H], residT[:], start=False, stop=True)
                z = tp.tile([H, w], f32, tag=f"z{w}")
                nc.scalar.activation(z[:], ps_z[:], ACT.Sigmoid, bias=brz[:, 1:2])

                x = tp.tile([H, w], f32, tag=f"x{w}")
                nc.vector.scalar_tensor_tensor(
                    x[:], ps_ghn[:], bhhn, r[:],
                    op0=AluOpType.add, op1=AluOpType.mult)   # x = (ghn+bhhn)*r

                ps_gxn = pp.tile([H, w], f32, tag="acc", bufs=4)
                nc.tensor.matmul(ps_gxn[:], vqp[:, 2 * H:3 * H], s4p[:, sl], start=True, stop=False)
                nc.tensor.matmul(ps_gxn[:], wih[:, 2 * H:3 * H], residT[:], start=False, stop=True)
                npre = tp.tile([H, w], f32, tag=f"npre{w}")
                nc.vector.tensor_add(npre[:], x[:], ps_gxn[:])
                nn = tp.tile([H, w], f32, tag=f"nn{w}")
                nc.scalar.activation(nn[:], npre[:], ACT.Tanh, bias=bihn)

                # out = n + z * (h - n); halves on separate engines,
                # one full-width store (fewer, larger descriptors)
                outc = op_.tile([H, w], f32, tag=f"outc{w}")
                for eng, cs in comb_eng:
                    cw = cs.stop - cs.start
                    osl = slice(sl.start + cs.start, sl.start + cs.stop)
                    d = tp.tile([H, cw], f32, tag=f"d{cw}{cs.start}")
                    eng.tensor_sub(d[:], hTr[:, osl].bitcast(f32), nn[:, cs])
                    e = tp.tile([H, cw], f32, tag=f"e{cw}{cs.start}")
                    eng.tensor_mul(e[:], z[:, cs], d[:])
                    eng.tensor_add(outc[:, cs], nn[:, cs], e[:])
                nc.scalar.dma_start(out_d[:, sl], outc[:])

            # One-quarter-behind pipeline: each gate block is emitted after
            # the NEXT msg accumulation, so a gate's residT copy always has
            # a full msg-block of slack and the PE never stalls on it.  The
            # last quarter is two column-split 256-wide halves, each fed by
            # its own A slab, so the serial gate tail runs on half tiles.
            def msg_q(q):
                return emit_msg(
                    NCHUNK, lambda k, q=q: (ats[(q, k // 8)], (k % 8) * NCHUNK),
                    f"psmsg{q}")

            def gates_q(q, ps):
                emit_gates(
                    slice(q * NCHUNK, (q + 1) * NCHUNK), NCHUNK, ps, False,
                    ((nc.vector, slice(0, 256)), (nc.gpsimd, slice(256, 512))),
                    f"residT{q}")

            for q in range(NCH - 1):
                gates_q(q, msg_q(q))
            psa = emit_msg(256, lambda k: (ats[(3, 0)], k * 256), "psmsg3a")
            psb = emit_msg(256, lambda k: (ats[(3, 1)], k * 256), "psmsg3b")
            emit_gates(slice(3 * NCHUNK, 3 * NCHUNK + 256), 256, psa, False,
                       ((nc.gpsimd, slice(0, 256)),), "residT3a")
            emit_gates(slice(3 * NCHUNK + 256, N), 256, psb, True,
                       ((nc.vector, slice(0, 256)),), "residT3b")

            # small trailing matmuls on alternating mini-psums: keep the PE
            # pipeline alive so the last gate psums' sem wake-ups (observed
            # ~1-2us lag after the PE's final instruction) stay instant
            wsrc = warm[:, 0:H // 2].bitcast(f16)
            for i in range(6):
                ps_d = pp.tile([H, H], f32, tag="tps", bufs=2, name=f"psd{i}")
                nc.tensor.matmul(ps_d[:], wsrc, wsrc, start=True, stop=True)

    nc.compile()
    return nc


def _get_program():
    if "nc" not in _CACHE:
        _CACHE["nc"] = _build_program()
    return _CACHE["nc"]


def _r32r(x):
    """Emulate the PE's f32r rounding: round-to-nearest at 11 mantissa bits."""
    u = np.asarray(x, np.float32).view(np.uint32)
    u2 = ((u.astype(np.uint64) + 0x800) & ~np.uint64(0xFFF)).astype(np.uint32)
    return u2.view(np.float32)


def _hl(w):
    hi = _r32r(w)
    return np.concatenate([hi, _r32r(w - hi)], axis=1)


def _make_in_maps(h, A, W1, b1, W2, b2, W_ih, W_hh, b_ih, b_hh):
    f = np.float32
    h = np.asarray(h, f); A = np.asarray(A)
    W1 = np.asarray(W1, f); W2 = np.asarray(W2, f)
    W_ih = np.asarray(W_ih, f); W_hh = np.asarray(W_hh, f)
    b1 = np.asarray(b1, f); b2 = np.asarray(b2, f)
    b_ih = np.asarray(b_ih, f); b_hh = np.asarray(b_hh, f)
    assert not np.any(b2), "kernel fuses relu-u assuming b2 == 0"
    assert not np.any(b1), "kernel computes the m1 relu without bias (b1 == 0)"

    w12hl = np.ascontiguousarray(
        np.concatenate([_hl(W1.T.astype(f)), _hl(W2.T.astype(f))], axis=1))
    sblg = np.zeros((H, C_G), dtype=f)
    sblg[:, G_WIH:G_WIH + 3 * H] = W_ih.T
    sblg[:, G_WHH:G_WHH + 3 * H] = W_hh.T
    sblf = np.zeros((H, C_F), dtype=f)
    sblf[:, F_BRZ] = (b_ih + b_hh)[0:H]
    sblf[:, F_BRZ + 1] = (b_ih + b_hh)[H:2 * H]
    sblf[:, F_BIHN] = b_ih[2 * H:3 * H]
    sblf[:, F_BHHN] = b_hh[2 * H:3 * H]
    ident = np.ascontiguousarray(np.eye(H, dtype=np.float16))

    in_maps = []
    for bi in range(B):
        m = {"w12hl": w12hl, "ident": ident}
        m["hT"] = np.ascontiguousarray(h[bi].T)
        A16 = A[bi].astype(np.float16)
        AT = np.ascontiguousarray(A16.T)                  # [2048 m, 2048 n] fp16
        A2 = (AT.reshape(KBLK // 8, 8, H, NCH, NCHUNK)    # [g, t, p, q, j]
                .transpose(3, 0, 2, 1, 4)                 # [q, g, p, t, j]
                .reshape(NCH, KBLK // 8, H, 8 * NCHUNK))
        A2 = np.ascontiguousarray(A2)
        # repack quarter 3 column-split: slab h holds ALL 16 k-blocks for
        # cols [1536+256h, 1536+256(h+1)): slab[p, k*256+j] = AT[128k+p, .]
        A3 = (AT[:, 3 * NCHUNK:].reshape(KBLK, H, 2, 256)   # [k, p, h, j]
                .transpose(2, 1, 0, 3)                      # [h, p, k, j]
                .reshape(2, H, KBLK * 256))
        A2[3] = A3
        m["A2"] = A2

        # u = column means of m2 (must be exactly fp16-representable: half
        # of m2 is 0 post-relu, so m2c = -u there and rounding that
        # constant would be a systematic error over the K=2048 msg sum)
        m1 = np.maximum(h[bi] @ W1.T + b1, 0)
        m2 = np.maximum(m1 @ W2.T + b2, 0)
        u = m2.mean(axis=0).astype(np.float16).astype(np.float64)   # [H]
        v = W_ih.astype(np.float64) @ u                   # [3H]
        # s must match what the PE accumulates: row-sums of the fp16 A
        s = A16.astype(np.float64).sum(axis=1)            # [N]

        v32 = v.astype(f); s32 = s.astype(f)
        vhi = _r32r(v32); vlo = _r32r(v32 - vhi)
        shi = _r32r(s32); slo = _r32r(s32 - shi)
        blg = sblg.copy()
        blg[0:4, G_VQ:G_VQ + 3 * H] = np.stack([vhi, vhi, vlo, vlo], axis=0)
        m["blg"] = np.ascontiguousarray(blg)
        blf = sblf.copy()
        blf[:, F_UP] = u.astype(f)
        m["blf"] = np.ascontiguousarray(blf)
        m["s4"] = np.ascontiguousarray(np.stack([shi, slo, shi, slo], axis=0))
        in_maps.append(m)
    return in_maps


def run(inputs, trace=False, trace_cores=None):
    """Build (cached), run on 8 cores, return (output, BassKernelResults)."""
    from concourse.bass_utils import run_bass_kernel_spmd

    nc = _get_program()
    in_maps = _make_in_maps(**inputs)
    res = run_bass_kernel_spmd(
        nc, in_maps, list(range(B)), trace=trace,
        trace_cores=trace_cores,
    )
    out = np.stack([res.results[b]["outT"].T for b in range(B)]).astype(np.float32)
    return out, res


def kernel(**inputs):
    out, _ = run(inputs, trace=False)
    return out


# revision 36
# speedup vs baseline: 1.0782x; 1.0782x over previous
"""Trainium2 Bass kernel for a GNN message-passing layer.

Reference computation (per batch b):
    m   = relu(h @ W1.T + b1)
    m   = relu(m @ W2.T + b2)
    msg = relu(A @ m)
    gx  = msg @ W_ih.T + b_ih ; gh = h @ W_hh.T + b_hh   (gates r,z,n)
    r = sig(gxr+ghr); z = sig(gxz+ghz); n = tanh(gxn + r*ghn)
    out = (1-z)*n + z*h
Sharding: pure data-parallel over B (B == n_cores == 8).

Numerics:
  * A streamed fp16; msg decomposed msg = u (x) s + A @ (m2 - u) with
    u ~= column means of m2 (fp16-exact), s = rowsums of fp16 A; the
    streamed residual is ~40x smaller than msg so gate matmuls run in
    fast f32r. v (x) s (v = W_ih @ u) restored via an exact hi/lo f32r
    matmul (128-padded stationary; K=4 matmuls measured 2x slower).
  * Weight rounding is a per-column systematic error that the ~1024x
    adjacency sum amplifies ~1% - so W1 and W2 are hi+lo f32r splits
    (exact) and only h/m1 DATA get f32r-rounded (dithered, safe).
Performance notes (all measured on traces):
  * One ring with the deep descriptor queue gets ~330-420GB/s; any
    competing ring starves.  sync ring = [w1, hT, all A] in need order;
    scalar ring = small blobs early + out stores late; big descriptors
    only (tiny ones melt aggregate bandwidth).
  * PE needs ~9.4us of busy time before f32r/f16 matmuls hit full rate
    (216ns per 512-col): throwaway f32 matmuls from ~8us warm it up.
  * m2 computed FEATURE-major (one stationary pair for all chunks, no
    16x f32 LDWEIGHTS) then transposed through the PE into node-major
    fp16 blocks for the msg stationary.
  * Last quarter's A is packed column-split so each 256-wide half
    finishes with its own slab: the serial sig/tanh/combine tail runs
    on half tiles and overlaps the other half's matmuls.
  * ACT sem wake-ups can lag ~2-3us behind a finishing PE; the q3
    half-split keeps the ACT queue busy so wakes stay instant.
"""

import numpy as np

B, N, H = 8, 2048, 128
NCHUNK = 512
NCH = N // NCHUNK  # 4
KBLK = N // 128    # 16

# blob G (f32r): gate weights + v-factors + W2 hi/lo split
G_WIH = 0          # [0:384)     W_ih.T
G_WHH = 384        # [384:768)   W_hh.T
G_VQ = 768         # [768:1152)  rows 0:4 = [vhi;vhi;vlo;vlo], rest 0
C_G = 1152
# blob F (f32): gate biases + u (per-partition)
F_BRZ = 0          # [0:2)
F_BIHN = 2
F_BHHN = 3
F_UP = 4
C_F = 5

_CACHE = {}


def _build_program():
    import concourse.bacc as bacc
    import concourse.tile as tile
    import concourse.mybir as mybir
    from concourse.alu_op_type import AluOpType

    f32 = mybir.dt.float32
    f32r = mybir.dt.float32r
    f16 = mybir.dt.float16
    ACT = mybir.ActivationFunctionType

    nc = bacc.Bacc("TRN2", target_bir_lowering=False, debug=False, num_devices=B)

    hT_d = nc.dram_tensor("hT", [H, N], f32r, kind="ExternalInput").ap()
    A2_d = nc.dram_tensor("A2", [NCH, KBLK // 8, H, 8 * NCHUNK], f16, kind="ExternalInput").ap()
    w1_d = nc.dram_tensor("w12hl", [H, 4 * H], f32r, kind="ExternalInput").ap()
    blg_d = nc.dram_tensor("blg", [H, C_G], f32r, kind="ExternalInput").ap()
    blf_d = nc.dram_tensor("blf", [H, C_F], f32, kind="ExternalInput").ap()
    id_d = nc.dram_tensor("ident", [H, H], f16, kind="ExternalInput").ap()
    s4_d = nc.dram_tensor("s4", [4, N], f32r, kind="ExternalInput").ap()
    out_d = nc.dram_tensor("outT", [H, N], f32, kind="ExternalOutput").ap()

    with tile.TileContext(nc) as tc:
        with (
            tc.tile_pool(name="consts", bufs=1) as cp,
            tc.tile_pool(name="big", bufs=1) as bp,
            tc.tile_pool(name="at", bufs=8) as ap_,
            tc.tile_pool(name="msgp", bufs=2) as mp,
            tc.tile_pool(name="tmp", bufs=2) as tp,
            tc.tile_pool(name="outp", bufs=2) as op_,
            tc.tile_pool(name="psum", bufs=1, space="PSUM") as pp,
        ):
            w12 = cp.tile([H, 4 * H], f32r, tag="w12")
            blg = cp.tile([H, C_G], f32r, tag="blg")
            blf = cp.tile([H, C_F], f32, tag="blf")
            ident = cp.tile([H, H], f16, tag="ident")
            dummy = cp.tile([H, 1], f32, tag="dummy")
            warm = cp.tile([H, 5 * H], f32, tag="warm")
            s4p = cp.tile([H, N], f32r, tag="s4p")
            hTr = bp.tile([H, N], f32r, tag="hTr")
            m1T = bp.tile([H, N], f32r, tag="m1T")
            m2cT = bp.tile([H, N], f16, tag="m2cT")  # feature-major (m2 - u)
            m2c = bp.tile([H, N], f16, tag="m2c")    # node-major, block k at 128k..

            wih = blg[:, G_WIH:G_WIH + 3 * H]
            whh = blg[:, G_WHH:G_WHH + 3 * H]
            vqp = blg[:, G_VQ:G_VQ + 3 * H]
            brz = blf[:, F_BRZ:F_BRZ + 2]
            bihn = blf[:, F_BIHN:F_BIHN + 1]
            bhhn = blf[:, F_BHHN:F_BHHN + 1]
            uP = blf[:, F_UP:F_UP + 1]

            # ---- PE warm-up from ~7.5us (gpsimd memset is the earliest
            # writer); ~9.4us of busy time until full matmul rate ----
            nc.gpsimd.memset(warm[:], 0.0)
            ps_w = pp.tile([H, NCHUNK], f32, tag="msg", bufs=2, name="pswarm")
            for _ in range(5):
                nc.tensor.matmul(ps_w[:], warm[:, 0:H], warm[:, H:5 * H],
                                 start=True, stop=True)

            # ---- DMA issues ----
            nc.sync.dma_start(w12[:], w1_d[:])
            nc.sync.dma_start(hTr[:], hT_d[:])
            ats = {}
            for q in range(NCH):
                for g_ in range(KBLK // 8):
                    at = ap_.tile([H, 8 * NCHUNK], f16, tag="at")
                    nc.sync.dma_start(at[:], A2_d[q, g_])
                    ats[(q, g_)] = at
            nc.scalar.dma_start(blf[:], blf_d[:])
            nc.scalar.dma_start(blg[:], blg_d[:])
            nc.scalar.dma_start(ident[:], id_d[:])
            nc.vector.memset(s4p[:].bitcast(f32), 0.0)
            nc.scalar.dma_start(s4p[0:4, :], s4_d[:])

            # ---- ACT table preload (dummy sigmoid; that table also holds
            # relu/tanh/copy) ----
            nc.vector.memset(dummy[:], 0.0)
            nc.scalar.activation(dummy[:], dummy[:], ACT.Sigmoid)

            # ---- m1T = relu(W1 @ hT), W1 split hi/lo f32r; relus
            # alternate DVE/ACT (b1 == 0 per spec) ----
            for c in range(NCH):
                sl = slice(c * NCHUNK, (c + 1) * NCHUNK)
                ps_m1 = pp.tile([H, NCHUNK], f32, tag="acc", bufs=4)
                nc.tensor.matmul(ps_m1[:], w12[:, 0:H], hTr[:, sl], start=True, stop=False)
                nc.tensor.matmul(ps_m1[:], w12[:, H:2 * H], hTr[:, sl], start=False, stop=True)
                if c % 2 == 0:
                    nc.vector.tensor_scalar_max(m1T[:, sl], ps_m1[:], 0.0)
                else:
                    nc.scalar.activation(m1T[:, sl], ps_m1[:], ACT.Relu)

            # ---- m2cT = relu(W2 @ m1) - u, FEATURE-major: W2 split hi/lo
            # f32r (exact), m1 f32r-rounded (dithered).  One fused DVE op
            # per chunk: (ps max 0) - u[p]  (b2 == 0 per spec) ----
            for c in range(NCH):
                sl = slice(c * NCHUNK, (c + 1) * NCHUNK)
                ps_m2 = pp.tile([H, NCHUNK], f32, tag="acc", bufs=4)
                nc.tensor.matmul(ps_m2[:], w12[:, 2 * H:3 * H], m1T[:, sl], start=True, stop=False)
                nc.tensor.matmul(ps_m2[:], w12[:, 3 * H:4 * H], m1T[:, sl], start=False, stop=True)
                nc.vector.tensor_scalar(m2cT[:, sl], ps_m2[:], 0.0, uP,
                                        op0=AluOpType.max, op1=AluOpType.subtract)

            # ---- transpose m2cT into node-major 128-blocks through the PE;
            # psum->SBUF copies alternate ACT/DVE ----
            for k in range(KBLK):
                kb = slice(k * H, (k + 1) * H)
                ps_t = pp.tile([H, H], f16, tag="tps", bufs=2)
                nc.tensor.transpose(ps_t[:], m2cT[:, kb], ident[:])
                if k % 2 == 0:
                    nc.vector.tensor_scalar_add(m2c[:, kb], ps_t[:], 0.0)
                else:
                    nc.scalar.copy(m2c[:, kb], ps_t[:])

            # ---- pipelined quarters ----
            def emit_msg(cols, src, name):
                # one psum accumulation over all 16 k-blocks, cols wide;
                # src(k) -> (at tile, column offset)
                ps_msg = pp.tile([H, cols], f32, tag="msg", bufs=2, name=name)
                for k in range(KBLK):
                    at, off = src(k)
                    nc.tensor.matmul(
                        ps_msg[:],
                        m2c[:, k * H:(k + 1) * H],
                        at[:, off:off + cols],
                        start=(k == 0), stop=(k == KBLK - 1),
                    )
                return ps_msg

            def emit_gates(sl, w, ps_msg, resid_dve, comb_eng, name):
                # sl: output column slice (width w)
                residT = mp.tile([H, w], f32r, tag=f"residT{w}", name=name)
                if resid_dve:
                    nc.vector.tensor_scalar_add(residT[:], ps_msg[:], 0.0)
                else:
                    nc.scalar.copy(residT[:], ps_msg[:])

                ps_ghn = pp.tile([H, w], f32, tag="acc", bufs=4)
                nc.tensor.matmul(ps_ghn[:], whh[:, 2 * H:3 * H], hTr[:, sl], start=True, stop=True)

                ps_r = pp.tile([H, w], f32, tag="acc", bufs=4)
                nc.tensor.matmul(ps_r[:], whh[:, 0:H], hTr[:, sl], start=True, stop=False)
                nc.tensor.matmul(ps_r[:], vqp[:, 0:H], s4p[:, sl], start=False, stop=False)
                nc.tensor.matmul(ps_r[:], wih[:, 0:H], residT[:], start=False, stop=True)
                r = tp.tile([H, w], f32, tag=f"r{w}")
                nc.scalar.activation(r[:], ps_r[:], ACT.Sigmoid, bias=brz[:, 0:1])

                ps_z = pp.tile([H, w], f32, tag="acc", bufs=4)
                nc.tensor.matmul(ps_z[:], whh[:, H:2 * H], hTr[:, sl], start=True, stop=False)
                nc.tensor.matmul(ps_z[:], vqp[:, H:2 * H], s4p[:, sl], start=False, stop=False)
                nc.tensor.matmul(ps_z[:], wih[:, H:2 * H], residT[:], start=False, stop=True)
                z = tp.tile([H, w], f32, tag=f"z{w}")
                nc.scalar.activation(z[:], ps_z[:], ACT.Sigmoid, bias=brz[:, 1:2])

                x = tp.tile([H, w], f32, tag=f"x{w}")
                nc.vector.scalar_tensor_tensor(
                    x[:], ps_ghn[:], bhhn, r[:],
                    op0=AluOpType.add, op1=AluOpType.mult)   # x = (ghn+bhhn)*r

                ps_gxn = pp.tile([H, w], f32, tag="acc", bufs=4)
                nc.tensor.matmul(ps_gxn[:], vqp[:, 2 * H:3 * H], s4p[:, sl], start=True, stop=False)
                nc.tensor.matmul(ps_gxn[:], wih[:, 2 * H:3 * H], residT[:], start=False, stop=True)
                npre = tp.tile([H, w], f32, tag=f"npre{w}")
                nc.vector.tensor_add(npre[:], x[:], ps_gxn[:])
                nn = tp.tile([H, w], f32, tag=f"nn{w}")
                nc.scalar.activation(nn[:], npre[:], ACT.Tanh, bias=bihn)

                # out = n + z * (h - n); halves on separate engines,
                # one full-width store (fewer, larger descriptors)
                outc = op_.tile([H, w], f32, tag=f"outc{w}")
                for eng, cs in comb_eng:
                    cw = cs.stop - cs.start
                    osl = slice(sl.start + cs.start, sl.start + cs.stop)
                    d = tp.tile([H, cw], f32, tag=f"d{cw}{cs.start}")
                    eng.tensor_sub(d[:], hTr[:, osl].bitcast(f32), nn[:, cs])
                    e = tp.tile([H, cw], f32, tag=f"e{cw}{cs.start}")
                    eng.tensor_mul(e[:], z[:, cs], d[:])
                    eng.tensor_add(outc[:, cs], nn[:, cs], e[:])
                nc.scalar.dma_start(out_d[:, sl], outc[:])

            # One-quarter-behind pipeline: each gate block is emitted after
            # the NEXT msg accumulation, so a gate's residT copy always has
            # a full msg-block of slack and the PE never stalls on it.  The
            # last quarter is two column-split 256-wide halves, each fed by
            # its own A slab, so the serial gate tail runs on half tiles.
            def msg_q(q):
                return emit_msg(
                    NCHUNK, lambda k, q=q: (ats[(q, k // 8)], (k % 8) * NCHUNK),
                    f"psmsg{q}")

            def gates_q(q, ps):
                emit_gates(
                    slice(q * NCHUNK, (q + 1) * NCHUNK), NCHUNK, ps, False,
                    ((nc.vector, slice(0, 256)), (nc.gpsimd, slice(256, 512))),
                    f"residT{q}")

            for q in range(NCH - 1):
                gates_q(q, msg_q(q))
            psa = emit_msg(256, lambda k: (ats[(3, 0)], k * 256), "psmsg3a")
            psb = emit_msg(256, lambda k: (ats[(3, 1)], k * 256), "psmsg3b")
            emit_gates(slice(3 * NCHUNK, 3 * NCHUNK + 256), 256, psa, False,
                       ((nc.gpsimd, slice(0, 256)),), "residT3a")
            emit_gates(slice(3 * NCHUNK + 256, N), 256, psb, True,
                       ((nc.vector, slice(0, 256)),), "residT3b")

    nc.compile()
    return nc


def _get_program():
    if "nc" not in _CACHE:
        _CACHE["nc"] = _build_program()
    return _CACHE["nc"]


def _r32r(x):
    """Emulate the PE's f32r rounding: round-to-nearest at 11 mantissa bits."""
    u = np.asarray(x, np.float32).view(np.uint32)
    u2 = ((u.astype(np.uint64) + 0x800) & ~np.uint64(0xFFF)).astype(np.uint32)
    return u2.view(np.float32)


def _hl(w):
    hi = _r32r(w)
    return np.concatenate([hi, _r32r(w - hi)], axis=1)


def _make_in_maps(h, A, W1, b1, W2, b2, W_ih, W_hh, b_ih, b_hh):
    f = np.float32
    h = np.asarray(h, f); A = np.asarray(A)
    W1 = np.asarray(W1, f); W2 = np.asarray(W2, f)
    W_ih = np.asarray(W_ih, f); W_hh = np.asarray(W_hh, f)
    b1 = np.asarray(b1, f); b2 = np.asarray(b2, f)
    b_ih = np.asarray(b_ih, f); b_hh = np.asarray(b_hh, f)
    assert not np.any(b2), "kernel fuses relu-u assuming b2 == 0"
    assert not np.any(b1), "kernel computes the m1 relu without bias (b1 == 0)"

    w12hl = np.ascontiguousarray(
        np.concatenate([_hl(W1.T.astype(f)), _hl(W2.T.astype(f))], axis=1))
    sblg = np.zeros((H, C_G), dtype=f)
    sblg[:, G_WIH:G_WIH + 3 * H] = W_ih.T
    sblg[:, G_WHH:G_WHH + 3 * H] = W_hh.T
    sblf = np.zeros((H, C_F), dtype=f)
    sblf[:, F_BRZ] = (b_ih + b_hh)[0:H]
    sblf[:, F_BRZ + 1] = (b_ih + b_hh)[H:2 * H]
    sblf[:, F_BIHN] = b_ih[2 * H:3 * H]
    sblf[:, F_BHHN] = b_hh[2 * H:3 * H]
    ident = np.ascontiguousarray(np.eye(H, dtype=np.float16))

    in_maps = []
    for bi in range(B):
        m = {"w12hl": w12hl, "ident": ident}
        m["hT"] = np.ascontiguousarray(h[bi].T)
        A16 = A[bi].astype(np.float16)
        AT = np.ascontiguousarray(A16.T)                  # [2048 m, 2048 n] fp16
        A2 = (AT.reshape(KBLK // 8, 8, H, NCH, NCHUNK)    # [g, t, p, q, j]
                .transpose(3, 0, 2, 1, 4)                 # [q, g, p, t, j]
                .reshape(NCH, KBLK // 8, H, 8 * NCHUNK))
        A2 = np.ascontiguousarray(A2)
        # repack quarter 3 column-split: slab h holds ALL 16 k-blocks for
        # cols [1536+256h, 1536+256(h+1)): slab[p, k*256+j] = AT[128k+p, .]
        A3 = (AT[:, 3 * NCHUNK:].reshape(KBLK, H, 2, 256)   # [k, p, h, j]
                .transpose(2, 1, 0, 3)                      # [h, p, k, j]
                .reshape(2, H, KBLK * 256))
        A2[3] = A3
        m["A2"] = A2

        # u = column means of m2 (must be exactly fp16-representable: half
        # of m2 is 0 post-relu, so m2c = -u there and rounding that
        # constant would be a systematic error over the K=2048 msg sum)
        m1 = np.maximum(h[bi] @ W1.T + b1, 0)
        m2 = np.maximum(m1 @ W2.T + b2, 0)
        u = m2.mean(axis=0).astype(np.float16).astype(np.float64)   # [H]
        v = W_ih.astype(np.float64) @ u                   # [3H]
        # s must match what the PE accumulates: row-sums of the fp16 A
        s = A16.astype(np.float64).sum(axis=1)            # [N]

        v32 = v.astype(f); s32 = s.astype(f)
        vhi = _r32r(v32); vlo = _r32r(v32 - vhi)
        shi = _r32r(s32); slo = _r32r(s32 - shi)
        blg = sblg.copy()
        blg[0:4, G_VQ:G_VQ + 3 * H] = np.stack([vhi, vhi, vlo, vlo], axis=0)
        m["blg"] = np.ascontiguousarray(blg)
        blf = sblf.copy()
        blf[:, F_UP] = u.astype(f)
        m["blf"] = np.ascontiguousarray(blf)
        m["s4"] = np.ascontiguousarray(np.stack([shi, slo, shi, slo], axis=0))
        in_maps.append(m)
    return in_maps


def run(inputs, trace=False, trace_cores=None):
    """Build (cached), run on 8 cores, return (output, BassKernelResults)."""
    from concourse.bass_utils import run_bass_kernel_spmd

    nc = _get_program()
    in_maps = _make_in_maps(**inputs)
    res = run_bass_kernel_spmd(
        nc, in_maps, list(range(B)), trace=trace,
        trace_cores=trace_cores,
    )
    out = np.stack([res.results[b]["outT"].T for b in range(B)]).astype(np.float32)
    return out, res


def kernel(**inputs):
    out, _ = run(inputs, trace=False)
    return out
